# revision 37
# baseline (speedup 1.0000x reference)
"""Trainium2 Bass kernel for nn_MemoryReader (retrieval_knn).

Math (per batch b, with softmax over the 4 heads):
  sim_h[n,m] = msn[n] * (sum_c -qe_h*mk_h^3 + 2qk_h*qe_h*mk_h - b_h[m]),
  aff = softmax_h(sim), mem[h,c',m] = sum_n mo[h,c',n] aff[h,n,m].

Difference-softmax form (exact): with d_h = sim_h - sim_0 for h=1..3,
  r = 1/(1 + sum_h exp(d_h)),  aff_0 = r,  aff_h = exp(d_h) * r.
Only THREE exps per (n,m) instead of four; aff_0 needs no multiply.

Each d_h is one K=65 bf16 matmul (bf16 keeps the PE's HAM activity
counter fed and enables FWL weight loads; fp32r sims left the PE cold
at 1.2 GHz): rows = [mk3_h*msn; mk_h*msn; mk3_0*msn; mk_0*msn; msn]
against w rows [-qe_h; 2qk_h*qe_h; +qe_0; -2qk_0*qe_0; (b_0-b_h)].

Sharding: 8 cores = 2 batches x 4 THW-chunks (n-chunk 2048/core). Softmax
over heads is core-local; readout partial sums over n are reduced on host.

Per-core dataflow, per (mh half of m, nt of 16 n-tiles):
  3 sim matmuls -> PSUM [128,1536] (3 banks, double-buffered = 6 banks)
  one Exp (ACT) -> e bf16 [128,1536]
  t = e1+e2, s = (t+1)+e3 (STT), r = recip_approx(s), aff_h = e_h*r --
  ALL on the DVE: routing any of these through GPSIMD head-blocks the
  DVE FIFO behind GpSimd's ~1us semaphore latency (measured +5-20us).
  r is written straight into the aff buffer: it IS head-0's affinity.
  readout: one PSUM bank per head-pass ([128,512], 2 slots); heads 0-1
  accumulate inside the nt loop, heads 2-3 replay from the persistent
  SBUF aff buffer afterwards (the aff buffer is double-buffered so the
  next mh's softmax never waits on this mh's deferred readouts).
"""

import sys

sys.path.insert(0, "/opt/trn_rl_repo")

import numpy as np

import concourse.bass as bass
import concourse.tile as tile
from concourse import bacc, mybir
from concourse.bass_utils import run_bass_kernel_spmd

try:
    import ml_dtypes

    _BF16_NP = np.dtype(ml_dtypes.bfloat16)
except ImportError:  # pragma: no cover
    _BF16_NP = None

HEADS, B, CK, CV = 4, 2, 64, 512
T, H, W = 8, 32, 32
THW, HW = T * H * W, H * W          # 8192, 1024
C = CK // HEADS                      # 16
NCHUNK = THW // 4                    # 2048 n per core
NT = NCHUNK // 128                   # 16 n-tiles per core
KD = 4 * C + 1                       # 65 rows of the diff matmul

F32 = mybir.dt.float32
F32R = mybir.dt.float32r
BF16 = mybir.dt.bfloat16

Add = mybir.AluOpType.add


def build_bass():
    nc = bacc.Bacc(None)
    # float32r must be produced as float32r (consumer-side bitcast rejected);
    # numpy side stays float32 (identical bits).
    xs_d = nc.dram_tensor("xs", [KD, 3 * NCHUNK], BF16, kind="ExternalInput")
    ws_d = nc.dram_tensor("ws", [KD, 3 * HW], BF16, kind="ExternalInput")
    mvt_d = nc.dram_tensor("mvt", [NCHUNK, CV], BF16, kind="ExternalInput")
    mem_d = nc.dram_tensor("mem", [CV, HW], F32, kind="ExternalOutput")

    Exp = mybir.ActivationFunctionType.Exp
    Copy = mybir.ActivationFunctionType.Copy

    from concourse.dve_ops import (
        RECIP_APPROX_FAST_CONSTS as _RC,
        RECIPROCAL_APPROX_FAST as _RF,
    )

    with tile.TileContext(nc) as tc:
        with (
            tc.tile_pool(name="const", bufs=1) as constp,
            tc.tile_pool(name="simp", bufs=2, space="PSUM") as simp,
            tc.tile_pool(name="memp", bufs=2, space="PSUM") as memp,
            tc.tile_pool(name="work", bufs=6) as work,
            tc.tile_pool(name="affp", bufs=2) as affp,
            tc.tile_pool(name="outp", bufs=4) as outp,
        ):
            ws_sb = constp.tile([128, 3 * HW], BF16)
            nc.sync.dma_start(out=ws_sb[:KD, :], in_=ws_d[:, :])
            xs_sb = constp.tile([128, 3 * NCHUNK], BF16)
            # tiny nt=0 chunks first so the first sims start ASAP, then the
            # next few tiles, then the rest
            FR = 4 * 128
            for h in range(3):
                nc.sync.dma_start(
                    out=xs_sb[:KD, h * NCHUNK : h * NCHUNK + 128],
                    in_=xs_d[:, h * NCHUNK : h * NCHUNK + 128],
                )
            for h in range(3):
                nc.sync.dma_start(
                    out=xs_sb[:KD, h * NCHUNK + 128 : h * NCHUNK + FR],
                    in_=xs_d[:, h * NCHUNK + 128 : h * NCHUNK + FR],
                )
            mvt_sb = constp.tile([128, NT * CV], BF16)
            for nt in range(4):
                nc.sync.dma_start(
                    out=mvt_sb[:, nt * CV : (nt + 1) * CV],
                    in_=mvt_d[nt * 128 : (nt + 1) * 128, :],
                )
            for h in range(3):
                nc.sync.dma_start(
                    out=xs_sb[:KD, h * NCHUNK + FR : (h + 1) * NCHUNK],
                    in_=xs_d[:, h * NCHUNK + FR : (h + 1) * NCHUNK],
                )
            for nt in range(4, NT):
                nc.sync.dma_start(
                    out=mvt_sb[:, nt * CV : (nt + 1) * CV],
                    in_=mvt_d[nt * 128 : (nt + 1) * 128, :],
                )

            # Dummy exp on a memset tile: forces the ~2.7us ACT table load
            # to happen during the input-DMA wait instead of serializing
            # before the first real exp.
            hz = constp.tile([128, 8], BF16)
            nc.vector.memset(hz[:], 0.0)
            hdst = constp.tile([128, 8], BF16)
            hexp = nc.scalar.activation(hdst[:], hz[:], Exp)
            hexp.ins.bass_priority = -100

            for mh in range(2):
                aff = affp.tile([128, NT * 2048], BF16, tag="aff")
                mems = [memp.tile([128, 512], F32, tag="mem", name=f"mem{p}") for p in range(2)]
                for nt0 in range(0, NT, 2):
                    es = []
                    s2 = work.tile([128, 1024], BF16, tag="s1", bufs=3)
                    for k in range(2):
                        nt = nt0 + k
                        st = simp.tile([128, 1536], F32, tag="sim")
                        for h in range(3):
                            nc.tensor.matmul(
                                st[:, h * 512 : (h + 1) * 512],
                                lhsT=xs_sb[:KD, h * NCHUNK + nt * 128 : h * NCHUNK + nt * 128 + 128],
                                rhs=ws_sb[:KD, h * HW + mh * 512 : h * HW + mh * 512 + 512],
                                start=True,
                                stop=True,
                            )
                        e = work.tile([128, 1536], BF16, tag="e", name=f"e{k}")
                        nc.scalar.activation(e[:], st[:], Exp)
                        t = work.tile([128, 512], BF16, tag="t", name=f"t{k}")
                        nc.vector.tensor_add(t[:], e[:, :512], e[:, 512:1024])
                        nc.vector.scalar_tensor_tensor(
                            s2[:, k * 512 : (k + 1) * 512],
                            t[:],
                            1.0,
                            e[:, 1024:1536],
                            Add,
                            Add,
                        )
                        es.append(e)
                    # ONE double-width reciprocal for the pair (the custom op
                    # is 1x-rate, so only its fixed cost matters -- halved
                    # here); the strided out drops r into both n-tiles'
                    # aff_0 slots
                    ab0 = nt0 * 2048
                    nc.vector._custom_dve(
                        _RF,
                        out=aff[:, ab0 : ab0 + 2560].rearrange(
                            "p (i m) -> p i m", m=512
                        )[:, 0::4, :],
                        in0=s2[:],
                        s0=_RC["s0"],
                        s1=_RC["s1"],
                        imm2=_RC["imm2"],
                    )
                    for k in range(2):
                        nt = nt0 + k
                        ab = nt * 2048
                        nc.vector.tensor_mul(
                            aff[:, ab + 512 : ab + 2048].rearrange(
                                "p (h m) -> p h m", h=3
                            ),
                            es[k].rearrange("p (h m) -> p h m", h=3),
                            aff[:, ab : ab + 512][:, None, :].to_broadcast(
                                (128, 3, 512)
                            ),
                        )
                        # head-0/1 readouts ride along with the nt loop
                        for p in range(2):
                            ro = nc.tensor.matmul(
                                mems[p][:],
                                lhsT=mvt_sb[:, nt * CV + p * 128 : nt * CV + p * 128 + 128],
                                rhs=aff[:, ab + p * 512 : ab + (p + 1) * 512],
                                start=(nt == 0),
                                stop=(nt == NT - 1),
                            )
                            ro.ins.bass_priority = 40 + p

                def flush(p, mp):
                    ms = outp.tile([128, 512], F32, tag="ms", name=f"ms{mh}{p}")
                    nc.scalar.activation(ms[:], mp[:], Copy)
                    nc.sync.dma_start(
                        out=mem_d[p * 128 : (p + 1) * 128, mh * 512 : (mh + 1) * 512],
                        in_=ms[:],
                    )

                flush(0, mems[0])
                flush(1, mems[1])
                for p in range(2, HEADS):
                    # deferred passes: aff for all nt is already in SBUF
                    mp = memp.tile([128, 512], F32, tag="mem")
                    for nt in range(NT):
                        ro = nc.tensor.matmul(
                            mp[:],
                            lhsT=mvt_sb[:, nt * CV + p * 128 : nt * CV + p * 128 + 128],
                            rhs=aff[:, nt * 2048 + p * 512 : nt * 2048 + (p + 1) * 512],
                            start=(nt == 0),
                            stop=(nt == NT - 1),
                        )
                        ro.ins.bass_priority = 50 + p
                    flush(p, mp)
    return nc


def host_decompose(mk, qk, ms, qe, mv):
    """Build the 8 per-core input dicts."""
    mk_f = np.asarray(mk, np.float32).reshape(B, CK, THW)
    mv_f = np.asarray(mv, np.float32).reshape(B, CV, THW)
    ms_f = np.asarray(ms, np.float32).reshape(B, THW)
    qk_h = np.asarray(qk, np.float32).reshape(B, HEADS, C, HW)
    qe_h = np.asarray(qe, np.float32).reshape(B, HEADS, C, HW)

    msn = ms_f / np.float32(np.sqrt(CK))                       # [B, THW]
    mk_h = mk_f.reshape(B, HEADS, C, THW)
    mk3_h = mk_h * mk_h * mk_h
    b_h = np.sum(qe_h * qk_h**3, axis=2)                       # [B, HEADS, HW]

    # xs [B, 65, 3, THW]: per diff-head (real head h+1)
    xs_all = np.empty((B, KD, 3, THW), np.float32)
    ws_all = np.empty((B, KD, 3, HW), np.float32)
    for h in range(3):
        rh = h + 1
        xs_all[:, 0:C, h] = mk3_h[:, rh]
        xs_all[:, C : 2 * C, h] = mk_h[:, rh]
        xs_all[:, 2 * C : 3 * C, h] = mk3_h[:, 0]
        xs_all[:, 3 * C : 4 * C, h] = mk_h[:, 0]
        xs_all[:, 4 * C, h] = 1.0
        ws_all[:, 0:C, h] = -qe_h[:, rh]
        ws_all[:, C : 2 * C, h] = 2.0 * qk_h[:, rh] * qe_h[:, rh]
        ws_all[:, 2 * C : 3 * C, h] = qe_h[:, 0]
        ws_all[:, 3 * C : 4 * C, h] = -2.0 * qk_h[:, 0] * qe_h[:, 0]
        ws_all[:, 4 * C, h] = b_h[:, 0] - b_h[:, rh]
    xs_all *= msn[:, None, None, :]

    in_maps = []
    for core in range(8):
        b, j = core // 4, core % 4
        sl = slice(j * NCHUNK, (j + 1) * NCHUNK)
        xs = np.ascontiguousarray(
            xs_all[b, :, :, sl].reshape(KD, 3 * NCHUNK)
        ).astype(_BF16_NP)
        ws = np.ascontiguousarray(ws_all[b].reshape(KD, 3 * HW)).astype(_BF16_NP)
        mvt = np.ascontiguousarray(mv_f[b, :, sl].T).astype(_BF16_NP)
        in_maps.append({"xs": xs, "ws": ws, "mvt": mvt})
    return in_maps


_NC_CACHE = None


def _get_nc():
    global _NC_CACHE
    if _NC_CACHE is None:
        nc = build_bass()
        if not nc.is_finalized():
            nc.finalize()
        _NC_CACHE = nc
    return _NC_CACHE


def kernel(mk, qk, ms, qe, mv, qv, _trace=False, _trace_kwargs=None):
    in_maps = host_decompose(mk, qk, ms, qe, mv)
    nc = _get_nc()
    res = run_bass_kernel_spmd(
        nc, in_maps, list(range(8)), trace=_trace, **(_trace_kwargs or {})
    )
    mem = np.zeros((B, CV, HW), np.float32)
    for core in range(8):
        mem[core // 4] += res.results[core]["mem"]
    out = np.concatenate(
        [mem.reshape(B, CV, H, W), np.asarray(qv, np.float32).reshape(B, CV, H, W)],
        axis=1,
    )
    if _trace:
        return out, res
    return out


# revision 38
# speedup vs baseline: 1.0148x; 1.0148x over previous
"""Trainium2 Bass kernel for nn_MemoryReader (retrieval_knn).

Math (per batch b, with softmax over the 4 heads):
  sim_h[n,m] = msn[n] * (sum_c -qe_h*mk_h^3 + 2qk_h*qe_h*mk_h - b_h[m]),
  aff = softmax_h(sim), mem[h,c',m] = sum_n mo[h,c',n] aff[h,n,m].

Difference-softmax form (exact): with d_h = sim_h - sim_0 for h=1..3,
  r = 1/(1 + sum_h exp(d_h)),  aff_0 = r,  aff_h = exp(d_h) * r.
Only THREE exps per (n,m) instead of four; aff_0 needs no multiply.

Each d_h is one K=65 bf16 matmul (bf16 keeps the PE's HAM activity
counter fed and enables FWL weight loads; fp32r sims left the PE cold
at 1.2 GHz): rows = [mk3_h*msn; mk_h*msn; mk3_0*msn; mk_0*msn; msn]
against w rows [-qe_h; 2qk_h*qe_h; +qe_0; -2qk_0*qe_0; (b_0-b_h)].

Sharding: 8 cores = 2 batches x 4 THW-chunks (n-chunk 2048/core). Softmax
over heads is core-local; readout partial sums over n are reduced on host.

Per-core dataflow, per (mh half of m, nt of 16 n-tiles):
  3 sim matmuls -> PSUM [128,1536] (3 banks, double-buffered = 6 banks)
  one Exp (ACT) -> e bf16 [128,1536]
  t = e1+e2, s = (t+1)+e3 (STT), r = recip_approx(s), aff_h = e_h*r --
  ALL on the DVE: routing any of these through GPSIMD head-blocks the
  DVE FIFO behind GpSimd's ~1us semaphore latency (measured +5-20us).
  r is written straight into the aff buffer: it IS head-0's affinity.
  readout: one PSUM bank per head-pass ([128,512], 2 slots); heads 0-1
  accumulate inside the nt loop, heads 2-3 replay from the persistent
  SBUF aff buffer afterwards (the aff buffer is double-buffered so the
  next mh's softmax never waits on this mh's deferred readouts).
"""

import sys

sys.path.insert(0, "/opt/trn_rl_repo")

import numpy as np

import concourse.bass as bass
import concourse.tile as tile
from concourse import bacc, mybir
from concourse.bass_utils import run_bass_kernel_spmd

try:
    import ml_dtypes

    _BF16_NP = np.dtype(ml_dtypes.bfloat16)
except ImportError:  # pragma: no cover
    _BF16_NP = None

HEADS, B, CK, CV = 4, 2, 64, 512
T, H, W = 8, 32, 32
THW, HW = T * H * W, H * W          # 8192, 1024
C = CK // HEADS                      # 16
NCHUNK = THW // 4                    # 2048 n per core
NT = NCHUNK // 128                   # 16 n-tiles per core
KD = 4 * C + 1                       # 65 rows of the diff matmul

F32 = mybir.dt.float32
F32R = mybir.dt.float32r
BF16 = mybir.dt.bfloat16

Add = mybir.AluOpType.add


def build_bass():
    nc = bacc.Bacc(None)
    # float32r must be produced as float32r (consumer-side bitcast rejected);
    # numpy side stays float32 (identical bits).
    xs_d = nc.dram_tensor("xs", [KD, 3 * NCHUNK], BF16, kind="ExternalInput")
    ws_d = nc.dram_tensor("ws", [KD, 3 * HW], BF16, kind="ExternalInput")
    mvt_d = nc.dram_tensor("mvt", [NCHUNK, CV], BF16, kind="ExternalInput")
    mem_d = nc.dram_tensor("mem", [CV, HW], F32, kind="ExternalOutput")

    Exp = mybir.ActivationFunctionType.Exp
    Copy = mybir.ActivationFunctionType.Copy

    from concourse.dve_ops import (
        RECIP_APPROX_FAST_CONSTS as _RC,
        RECIPROCAL_APPROX_FAST as _RF,
    )

    with tile.TileContext(nc) as tc:
        with (
            tc.tile_pool(name="const", bufs=1) as constp,
            tc.tile_pool(name="simp", bufs=2, space="PSUM") as simp,
            tc.tile_pool(name="memp", bufs=2, space="PSUM") as memp,
            tc.tile_pool(name="work", bufs=6) as work,
            tc.tile_pool(name="affp", bufs=2) as affp,
            tc.tile_pool(name="outp", bufs=4) as outp,
        ):
            ws_sb = constp.tile([128, 3 * HW], BF16)
            nc.sync.dma_start(out=ws_sb[:KD, :], in_=ws_d[:, :])
            xs_sb = constp.tile([128, 3 * NCHUNK], BF16)
            # tiny nt=0 chunks first so the first sims start ASAP, then the
            # next few tiles, then the rest
            FR = 4 * 128
            for h in range(3):
                nc.sync.dma_start(
                    out=xs_sb[:KD, h * NCHUNK : h * NCHUNK + 128],
                    in_=xs_d[:, h * NCHUNK : h * NCHUNK + 128],
                )
            for h in range(3):
                nc.sync.dma_start(
                    out=xs_sb[:KD, h * NCHUNK + 128 : h * NCHUNK + FR],
                    in_=xs_d[:, h * NCHUNK + 128 : h * NCHUNK + FR],
                )
            mvt_sb = constp.tile([128, NT * CV], BF16)
            for nt in range(4):
                nc.sync.dma_start(
                    out=mvt_sb[:, nt * CV : (nt + 1) * CV],
                    in_=mvt_d[nt * 128 : (nt + 1) * 128, :],
                )
            for h in range(3):
                nc.sync.dma_start(
                    out=xs_sb[:KD, h * NCHUNK + FR : (h + 1) * NCHUNK],
                    in_=xs_d[:, h * NCHUNK + FR : (h + 1) * NCHUNK],
                )
            for nt in range(4, NT):
                nc.sync.dma_start(
                    out=mvt_sb[:, nt * CV : (nt + 1) * CV],
                    in_=mvt_d[nt * 128 : (nt + 1) * 128, :],
                )

            # Dummy exp on a memset tile: forces the ~2.7us ACT table load
            # to happen during the input-DMA wait instead of serializing
            # before the first real exp.
            hz = constp.tile([128, 8], BF16)
            nc.vector.memset(hz[:], 0.0)
            hdst = constp.tile([128, 8], BF16)
            hexp = nc.scalar.activation(hdst[:], hz[:], Exp)
            hexp.ins.bass_priority = -100

            for mh in range(2):
                aff = affp.tile([128, NT * 2048], BF16, tag="aff")
                mems = [memp.tile([128, 512], F32, tag="mem", name=f"mem{p}") for p in range(2)]
                for nt in range(NT):
                    st = simp.tile([128, 1536], F32, tag="sim")
                    for h in range(3):
                        nc.tensor.matmul(
                            st[:, h * 512 : (h + 1) * 512],
                            lhsT=xs_sb[:KD, h * NCHUNK + nt * 128 : h * NCHUNK + nt * 128 + 128],
                            rhs=ws_sb[:KD, h * HW + mh * 512 : h * HW + mh * 512 + 512],
                            start=True,
                            stop=True,
                        )
                    e = work.tile([128, 1536], BF16, tag="e")
                    nc.scalar.activation(e[:], st[:], Exp)
                    t = work.tile([128, 512], BF16, tag="t")
                    nc.vector.tensor_add(t[:], e[:, :512], e[:, 512:1024])
                    s1 = work.tile([128, 512], BF16, tag="s1")
                    nc.vector.scalar_tensor_tensor(
                        s1[:], t[:], 1.0, e[:, 1024:1536], Add, Add
                    )
                    ab = nt * 2048
                    # r = 1/(1+sum e) in bf16, written in place as aff_0
                    nc.vector._custom_dve(
                        _RF,
                        out=aff[:, ab : ab + 512],
                        in0=s1[:],
                        s0=_RC["s0"],
                        s1=_RC["s1"],
                        imm2=_RC["imm2"],
                    )
                    nc.vector.tensor_mul(
                        aff[:, ab + 512 : ab + 2048].rearrange(
                            "p (h m) -> p h m", h=3
                        ),
                        e.rearrange("p (h m) -> p h m", h=3),
                        aff[:, ab : ab + 512][:, None, :].to_broadcast(
                            (128, 3, 512)
                        ),
                    )
                    # head-0/1 readouts ride along with the nt loop
                    for p in range(2):
                        ro = nc.tensor.matmul(
                            mems[p][:],
                            lhsT=mvt_sb[:, nt * CV + p * 128 : nt * CV + p * 128 + 128],
                            rhs=aff[:, ab + p * 512 : ab + (p + 1) * 512],
                            start=(nt == 0),
                            stop=(nt == NT - 1),
                        )
                        ro.ins.bass_priority = 40 + p

                def flush(p, mp):
                    ms = outp.tile([128, 512], F32, tag="ms", name=f"ms{mh}{p}")
                    nc.scalar.activation(ms[:], mp[:], Copy)
                    nc.sync.dma_start(
                        out=mem_d[p * 128 : (p + 1) * 128, mh * 512 : (mh + 1) * 512],
                        in_=ms[:],
                    )

                flush(0, mems[0])
                flush(1, mems[1])
                for p in range(2, HEADS):
                    # deferred passes: aff for all nt is already in SBUF
                    mp = memp.tile([128, 512], F32, tag="mem")
                    for nt in range(NT):
                        ro = nc.tensor.matmul(
                            mp[:],
                            lhsT=mvt_sb[:, nt * CV + p * 128 : nt * CV + p * 128 + 128],
                            rhs=aff[:, nt * 2048 + p * 512 : nt * 2048 + (p + 1) * 512],
                            start=(nt == 0),
                            stop=(nt == NT - 1),
                        )
                        ro.ins.bass_priority = 50 + p
                    flush(p, mp)
    return nc


def host_decompose(mk, qk, ms, qe, mv):
    """Build the 8 per-core input dicts."""
    mk_f = np.asarray(mk, np.float32).reshape(B, CK, THW)
    mv_f = np.asarray(mv, np.float32).reshape(B, CV, THW)
    ms_f = np.asarray(ms, np.float32).reshape(B, THW)
    qk_h = np.asarray(qk, np.float32).reshape(B, HEADS, C, HW)
    qe_h = np.asarray(qe, np.float32).reshape(B, HEADS, C, HW)

    msn = ms_f / np.float32(np.sqrt(CK))                       # [B, THW]
    mk_h = mk_f.reshape(B, HEADS, C, THW)
    mk3_h = mk_h * mk_h * mk_h
    b_h = np.sum(qe_h * qk_h**3, axis=2)                       # [B, HEADS, HW]

    # xs [B, 65, 3, THW]: per diff-head (real head h+1)
    xs_all = np.empty((B, KD, 3, THW), np.float32)
    ws_all = np.empty((B, KD, 3, HW), np.float32)
    for h in range(3):
        rh = h + 1
        xs_all[:, 0:C, h] = mk3_h[:, rh]
        xs_all[:, C : 2 * C, h] = mk_h[:, rh]
        xs_all[:, 2 * C : 3 * C, h] = mk3_h[:, 0]
        xs_all[:, 3 * C : 4 * C, h] = mk_h[:, 0]
        xs_all[:, 4 * C, h] = 1.0
        ws_all[:, 0:C, h] = -qe_h[:, rh]
        ws_all[:, C : 2 * C, h] = 2.0 * qk_h[:, rh] * qe_h[:, rh]
        ws_all[:, 2 * C : 3 * C, h] = qe_h[:, 0]
        ws_all[:, 3 * C : 4 * C, h] = -2.0 * qk_h[:, 0] * qe_h[:, 0]
        ws_all[:, 4 * C, h] = b_h[:, 0] - b_h[:, rh]
    xs_all *= msn[:, None, None, :]

    in_maps = []
    for core in range(8):
        b, j = core // 4, core % 4
        sl = slice(j * NCHUNK, (j + 1) * NCHUNK)
        xs = np.ascontiguousarray(
            xs_all[b, :, :, sl].reshape(KD, 3 * NCHUNK)
        ).astype(_BF16_NP)
        ws = np.ascontiguousarray(ws_all[b].reshape(KD, 3 * HW)).astype(_BF16_NP)
        mvt = np.ascontiguousarray(mv_f[b, :, sl].T).astype(_BF16_NP)
        in_maps.append({"xs": xs, "ws": ws, "mvt": mvt})
    return in_maps


_NC_CACHE = None


def _get_nc():
    global _NC_CACHE
    if _NC_CACHE is None:
        nc = build_bass()
        if not nc.is_finalized():
            nc.finalize()
        _NC_CACHE = nc
    return _NC_CACHE


def kernel(mk, qk, ms, qe, mv, qv, _trace=False, _trace_kwargs=None):
    in_maps = host_decompose(mk, qk, ms, qe, mv)
    nc = _get_nc()
    res = run_bass_kernel_spmd(
        nc, in_maps, list(range(8)), trace=_trace, **(_trace_kwargs or {})
    )
    mem = np.zeros((B, CV, HW), np.float32)
    for core in range(8):
        mem[core // 4] += res.results[core]["mem"]
    out = np.concatenate(
        [mem.reshape(B, CV, H, W), np.asarray(qv, np.float32).reshape(B, CV, H, W)],
        axis=1,
    )
    if _trace:
        return out, res
    return out


# revision 39
# speedup vs baseline: 1.0474x; 1.0321x over previous
"""Trainium2 Bass kernel for nn_MemoryReader (retrieval_knn).

Math (per batch b, with softmax over the 4 heads):
  sim_h[n,m] = msn[n] * (sum_c -qe_h*mk_h^3 + 2qk_h*qe_h*mk_h - b_h[m]),
  aff = softmax_h(sim), mem[h,c',m] = sum_n mo[h,c',n] aff[h,n,m].

Difference-softmax form (exact): with d_h = sim_h - sim_0 for h=1..3,
  r = 1/(1 + sum_h exp(d_h)),  aff_0 = r,  aff_h = exp(d_h) * r.
Only THREE exps per (n,m) instead of four; aff_0 needs no multiply.

Each d_h is one K=65 bf16 matmul (bf16 keeps the PE's HAM activity
counter fed and enables FWL weight loads; fp32r sims left the PE cold
at 1.2 GHz): rows = [mk3_h*msn; mk_h*msn; mk3_0*msn; mk_0*msn; msn]
against w rows [-qe_h; 2qk_h*qe_h; +qe_0; -2qk_0*qe_0; (b_0-b_h)].

Sharding: 8 cores = 2 batches x 4 THW-chunks (n-chunk 2048/core). Softmax
over heads is core-local; readout partial sums over n are reduced on host.

Per-core dataflow, per (mh half of m, nt of 16 n-tiles):
  3 sim matmuls -> PSUM [128,1536] (3 banks, double-buffered = 6 banks)
  one Exp (ACT) -> e bf16 [128,1536]
  t = e1+e2, s = (t+1)+e3 (STT), r = recip_approx(s), aff_h = e_h*r --
  ALL on the DVE: routing any of these through GPSIMD head-blocks the
  DVE FIFO behind GpSimd's ~1us semaphore latency (measured +5-20us).
  r is written straight into the aff buffer: it IS head-0's affinity.
  readout: one PSUM bank per head-pass ([128,512], 2 slots); heads 0-1
  accumulate inside the nt loop, heads 2-3 replay from the persistent
  SBUF aff buffer afterwards (the aff buffer is double-buffered so the
  next mh's softmax never waits on this mh's deferred readouts).
"""

import sys

sys.path.insert(0, "/opt/trn_rl_repo")

import numpy as np

import concourse.bass as bass
import concourse.tile as tile
from concourse import bacc, mybir
from concourse.bass_utils import run_bass_kernel_spmd

try:
    import ml_dtypes

    _BF16_NP = np.dtype(ml_dtypes.bfloat16)
except ImportError:  # pragma: no cover
    _BF16_NP = None

HEADS, B, CK, CV = 4, 2, 64, 512
T, H, W = 8, 32, 32
THW, HW = T * H * W, H * W          # 8192, 1024
C = CK // HEADS                      # 16
NCHUNK = THW // 4                    # 2048 n per core
NT = NCHUNK // 128                   # 16 n-tiles per core
KD = 4 * C + 1                       # 65 rows of the diff matmul

F32 = mybir.dt.float32
F32R = mybir.dt.float32r
BF16 = mybir.dt.bfloat16

Add = mybir.AluOpType.add


def build_bass():
    nc = bacc.Bacc(None)
    # float32r must be produced as float32r (consumer-side bitcast rejected);
    # numpy side stays float32 (identical bits).
    xs_d = nc.dram_tensor("xs", [KD, 3 * NCHUNK], BF16, kind="ExternalInput")
    ws_d = nc.dram_tensor("ws", [KD, 3 * HW], BF16, kind="ExternalInput")
    mvt_d = nc.dram_tensor("mvt", [NCHUNK, CV], BF16, kind="ExternalInput")
    mem_d = nc.dram_tensor("mem", [CV, HW], F32, kind="ExternalOutput")

    Exp = mybir.ActivationFunctionType.Exp
    Copy = mybir.ActivationFunctionType.Copy

    from concourse.dve_ops import (
        RECIP_APPROX_FAST_CONSTS as _RC,
        RECIPROCAL_APPROX_FAST as _RF,
    )

    with tile.TileContext(nc) as tc:
        with (
            tc.tile_pool(name="const", bufs=1) as constp,
            tc.tile_pool(name="simp", bufs=2, space="PSUM") as simp,
            tc.tile_pool(name="memp", bufs=2, space="PSUM") as memp,
            tc.tile_pool(name="work", bufs=6) as work,
            tc.tile_pool(name="affp", bufs=2) as affp,
            tc.tile_pool(name="outp", bufs=4) as outp,
        ):
            ws_sb = constp.tile([128, 3 * HW], BF16)
            nc.sync.dma_start(out=ws_sb[:KD, :], in_=ws_d[:, :])
            xs_sb = constp.tile([128, 3 * NCHUNK], BF16)
            # tiny nt=0 chunks first so the first sims start ASAP, then the
            # next few tiles, then the rest
            FR = 4 * 128
            for h in range(3):
                nc.sync.dma_start(
                    out=xs_sb[:KD, h * NCHUNK : h * NCHUNK + 128],
                    in_=xs_d[:, h * NCHUNK : h * NCHUNK + 128],
                )
            for h in range(3):
                nc.sync.dma_start(
                    out=xs_sb[:KD, h * NCHUNK + 128 : h * NCHUNK + FR],
                    in_=xs_d[:, h * NCHUNK + 128 : h * NCHUNK + FR],
                )
            mvt_sb = constp.tile([128, NT * CV], BF16)
            for nt in range(4):
                nc.sync.dma_start(
                    out=mvt_sb[:, nt * CV : (nt + 1) * CV],
                    in_=mvt_d[nt * 128 : (nt + 1) * 128, :],
                )
            for h in range(3):
                nc.sync.dma_start(
                    out=xs_sb[:KD, h * NCHUNK + FR : (h + 1) * NCHUNK],
                    in_=xs_d[:, h * NCHUNK + FR : (h + 1) * NCHUNK],
                )
            for nt in range(4, NT):
                nc.sync.dma_start(
                    out=mvt_sb[:, nt * CV : (nt + 1) * CV],
                    in_=mvt_d[nt * 128 : (nt + 1) * 128, :],
                )

            # Dummy exp on a memset tile: forces the ~2.7us ACT table load
            # to happen during the input-DMA wait instead of serializing
            # before the first real exp.
            hz = constp.tile([128, 8], BF16)
            nc.vector.memset(hz[:], 0.0)
            hdst = constp.tile([128, 8], BF16)
            hexp = nc.scalar.activation(hdst[:], hz[:], Exp)
            hexp.ins.bass_priority = -100

            for mh in range(2):
                aff = affp.tile([128, NT * 2048], BF16, tag="aff")
                mems = [memp.tile([128, 512], F32, tag="mem", name=f"mem{p}") for p in range(2)]
                def tail_ops(nt, e, s1):
                    ab = nt * 2048
                    # r = 1/(1+sum e) in bf16, written in place as aff_0
                    nc.vector._custom_dve(
                        _RF,
                        out=aff[:, ab : ab + 512],
                        in0=s1[:],
                        s0=_RC["s0"],
                        s1=_RC["s1"],
                        imm2=_RC["imm2"],
                    )
                    nc.vector.tensor_mul(
                        aff[:, ab + 512 : ab + 2048].rearrange(
                            "p (h m) -> p h m", h=3
                        ),
                        e.rearrange("p (h m) -> p h m", h=3),
                        aff[:, ab : ab + 512][:, None, :].to_broadcast(
                            (128, 3, 512)
                        ),
                    )
                    # head-0/1 readouts ride along with the nt loop
                    for p in range(2):
                        ro = nc.tensor.matmul(
                            mems[p][:],
                            lhsT=mvt_sb[:, nt * CV + p * 128 : nt * CV + p * 128 + 128],
                            rhs=aff[:, ab + p * 512 : ab + (p + 1) * 512],
                            start=(nt == 0),
                            stop=(nt == NT - 1),
                        )
                        ro.ins.bass_priority = 40 + p

                # Software-pipelined by one iteration: the scalar "+1" runs
                # as an ACT Copy-with-bias (frees the 1x-rate STT into a 2x
                # tensor_add on the DVE), and the recip/mul/readouts of
                # iteration i-1 are emitted during iteration i so every op's
                # inputs are long-ready when it reaches its FIFO head --
                # no cross-engine head-blocking.
                prev = None
                for nt in range(NT):
                    st = simp.tile([128, 1536], F32, tag="sim")
                    for h in range(3):
                        nc.tensor.matmul(
                            st[:, h * 512 : (h + 1) * 512],
                            lhsT=xs_sb[:KD, h * NCHUNK + nt * 128 : h * NCHUNK + nt * 128 + 128],
                            rhs=ws_sb[:KD, h * HW + mh * 512 : h * HW + mh * 512 + 512],
                            start=True,
                            stop=True,
                        )
                    e = work.tile([128, 1536], BF16, tag="e")
                    nc.scalar.activation(e[:], st[:], Exp)
                    if prev is not None:
                        nc.scalar.activation(prev[3][:], prev[2][:], Copy, bias=1.0)
                    t = work.tile([128, 512], BF16, tag="t")
                    nc.vector.tensor_add(t[:], e[:, :512], e[:, 512:1024])
                    u = work.tile([128, 512], BF16, tag="u")
                    nc.vector.tensor_add(u[:], t[:], e[:, 1024:1536])
                    s1 = work.tile([128, 512], BF16, tag="s1")
                    if prev is not None:
                        tail_ops(prev[0], prev[1], prev[3])
                    prev = (nt, e, u, s1)
                nc.scalar.activation(prev[3][:], prev[2][:], Copy, bias=1.0)
                tail_ops(prev[0], prev[1], prev[3])

                def flush(p, mp):
                    ms = outp.tile([128, 512], F32, tag="ms", name=f"ms{mh}{p}")
                    nc.scalar.activation(ms[:], mp[:], Copy)
                    nc.sync.dma_start(
                        out=mem_d[p * 128 : (p + 1) * 128, mh * 512 : (mh + 1) * 512],
                        in_=ms[:],
                    )

                flush(0, mems[0])
                flush(1, mems[1])
                for p in range(2, HEADS):
                    # deferred passes: aff for all nt is already in SBUF
                    mp = memp.tile([128, 512], F32, tag="mem")
                    for nt in range(NT):
                        ro = nc.tensor.matmul(
                            mp[:],
                            lhsT=mvt_sb[:, nt * CV + p * 128 : nt * CV + p * 128 + 128],
                            rhs=aff[:, nt * 2048 + p * 512 : nt * 2048 + (p + 1) * 512],
                            start=(nt == 0),
                            stop=(nt == NT - 1),
                        )
                        ro.ins.bass_priority = 50 + p
                    flush(p, mp)
    return nc


def host_decompose(mk, qk, ms, qe, mv):
    """Build the 8 per-core input dicts."""
    mk_f = np.asarray(mk, np.float32).reshape(B, CK, THW)
    mv_f = np.asarray(mv, np.float32).reshape(B, CV, THW)
    ms_f = np.asarray(ms, np.float32).reshape(B, THW)
    qk_h = np.asarray(qk, np.float32).reshape(B, HEADS, C, HW)
    qe_h = np.asarray(qe, np.float32).reshape(B, HEADS, C, HW)

    msn = ms_f / np.float32(np.sqrt(CK))                       # [B, THW]
    mk_h = mk_f.reshape(B, HEADS, C, THW)
    mk3_h = mk_h * mk_h * mk_h
    b_h = np.sum(qe_h * qk_h**3, axis=2)                       # [B, HEADS, HW]

    # xs [B, 65, 3, THW]: per diff-head (real head h+1)
    xs_all = np.empty((B, KD, 3, THW), np.float32)
    ws_all = np.empty((B, KD, 3, HW), np.float32)
    for h in range(3):
        rh = h + 1
        xs_all[:, 0:C, h] = mk3_h[:, rh]
        xs_all[:, C : 2 * C, h] = mk_h[:, rh]
        xs_all[:, 2 * C : 3 * C, h] = mk3_h[:, 0]
        xs_all[:, 3 * C : 4 * C, h] = mk_h[:, 0]
        xs_all[:, 4 * C, h] = 1.0
        ws_all[:, 0:C, h] = -qe_h[:, rh]
        ws_all[:, C : 2 * C, h] = 2.0 * qk_h[:, rh] * qe_h[:, rh]
        ws_all[:, 2 * C : 3 * C, h] = qe_h[:, 0]
        ws_all[:, 3 * C : 4 * C, h] = -2.0 * qk_h[:, 0] * qe_h[:, 0]
        ws_all[:, 4 * C, h] = b_h[:, 0] - b_h[:, rh]
    xs_all *= msn[:, None, None, :]

    in_maps = []
    for core in range(8):
        b, j = core // 4, core % 4
        sl = slice(j * NCHUNK, (j + 1) * NCHUNK)
        xs = np.ascontiguousarray(
            xs_all[b, :, :, sl].reshape(KD, 3 * NCHUNK)
        ).astype(_BF16_NP)
        ws = np.ascontiguousarray(ws_all[b].reshape(KD, 3 * HW)).astype(_BF16_NP)
        mvt = np.ascontiguousarray(mv_f[b, :, sl].T).astype(_BF16_NP)
        in_maps.append({"xs": xs, "ws": ws, "mvt": mvt})
    return in_maps


_NC_CACHE = None


def _get_nc():
    global _NC_CACHE
    if _NC_CACHE is None:
        nc = build_bass()
        if not nc.is_finalized():
            nc.finalize()
        _NC_CACHE = nc
    return _NC_CACHE


def kernel(mk, qk, ms, qe, mv, qv, _trace=False, _trace_kwargs=None):
    in_maps = host_decompose(mk, qk, ms, qe, mv)
    nc = _get_nc()
    res = run_bass_kernel_spmd(
        nc, in_maps, list(range(8)), trace=_trace, **(_trace_kwargs or {})
    )
    mem = np.zeros((B, CV, HW), np.float32)
    for core in range(8):
        mem[core // 4] += res.results[core]["mem"]
    out = np.concatenate(
        [mem.reshape(B, CV, H, W), np.asarray(qv, np.float32).reshape(B, CV, H, W)],
        axis=1,
    )
    if _trace:
        return out, res
    return out
